# revision 16
# baseline (speedup 1.0000x reference)
"""AUGRU cell (attention-gated GRU update) on 8 Trainium2 NeuronCores.

Data-parallel: the batch dim (16384) of x / att_score / hidden is sharded
across 8 cores (2048 rows each); the six 512x512 weight matrices are
replicated.

Per-core dataflow (per 128-row batch tile, 16 tiles):
  zr = x @ W_r + h @ U_r          (PSUM accum, 8 matmuls)
  hu = h @ U_h ; xh = x @ W_h
  zu = x @ W_u + h @ U_u          (update gate last: shortest tail)
  r = sigmoid(zr); g = tanh(r * hu + xh); d = g - h
  u2 = att * sigmoid(zu)
  out = h + u2 * d                (== (1-u2)*h + u2*g)

The early phase is DMA-supply-bound: ~4.5MB (weights + first chunks)
must land in the first ~13us of queue time (3 queues x ~110 GB/s). The
START is therefore STAGED: tiles 0-3 run only their zr+hu groups first
(needs just wr/ur/uh = 1.5MB), and the xh/zu groups for tiles 0-3 run
afterwards, by which time wh/wu/uu have streamed in. Tiles 4+ run
tile-major (all weights resident). The epilogue tail (ep_rest) of tile t
is emitted after tile t+1's matmuls so ACT/DVE never head-of-line block
a PSUM-bank release.

Matmuls in bf16 (rel err ~3.6e-3 vs the 2e-2 gate; fp8 DoubleRow was
measured at 2.3-3.8e-2 on CPU — over the gate — so bf16's 82us PE floor
stands). All matmul operands are HOST-PREPACKED into DRAM buffers whose
per-partition bytes are contiguous in exactly the SBUF-resident layout.
h is loaded twice: K-major ("hT") for the PE and batch-major ("hN",
4 resident chunks, deferred — only DVE consumes it) for the epilogue.

Each weight travels in pieces matching PE consumption granularity
(wr/ur: ko-quarters, uh/wh/wu/uu: ko-halves) with one semaphore per
piece, spread EDF-style over the three DGE queues in consumption order
(the queues round-robin packets across outstanding transfers):
  sync:   xc0, ur q0-3, xc1, wu_h1, uu_h0, hN c0.., xc2.., out stores
  scalar: wr q0-3, wh_h0, wu_h0, uu_h1      then per-tile ACTs
  gpsimd: hc0, uh_h0, uh_h1, hc1, wh_h1, hc2.., att
Junk bf16 ldweights+matmuls (memset operands) keep the PE busy from
~7.5us until the first real operands land (~10.5us): the HAM clock gate
needs ~3.4us of sustained PE activity to lift the array to 2.4 GHz, and
an idle gap resets it. The LAST tile's zu runs as two column-half groups
in two separate half-bank PSUM tiles (Tile deps are tile-granular, so
separate tiles are needed for the overlap), letting half 0's
sigmoid/stt/add/store overlap half 1's matmuls — only a 256-wide chain
trails the final matmul. PSUM banks keep a single releasing engine
(zu/zr: ACT sigmoid; hu/xh: DVE); stray multi-waits are legalized by
_split_multi_waits.
"""

import os
import sys

if "/opt/trn_rl_repo" not in sys.path:
    sys.path.insert(0, "/opt/trn_rl_repo")

import numpy as np

NCORES = 8
P = 128
MM_DTYPE = os.environ.get("MM_DTYPE", "bf16")  # "bf16" or "f32r"
N_JUNK_LDW = int(os.environ.get("N_JUNK_LDW", "6"))
N_JUNK_MM = int(os.environ.get("N_JUNK_MM", "10"))


def _bchunks(Bc):
    """Batch-chunk widths for the x/h loads: two small first chunks
    covering the staged first four tiles, then wide chunks."""
    ws, rem = [], Bc
    for w in (2 * P, 2 * P):
        if rem <= 0:
            break
        w = min(w, rem)
        ws.append(w)
        rem -= w
    while rem > 0:
        w = min(4 * P, rem)
        ws.append(w)
        rem -= w
    return ws


def _hchunks(tiles):
    """hN chunk widths in tiles (4 chunks of 4 for the 16-tile case)."""
    ws, rem = [], tiles
    while rem > 0:
        w = min(4, rem)
        ws.append(w)
        rem -= w
    return ws


_PROGRAM_CACHE = {}


def _split_multi_waits(nc):
    """walrus codegen accepts at most ONE sync wait per instruction (the
    TPB EVENTS struct has a single wait slot and setupSyncWait refuses to
    spill).  Tile's add_semaphores can emit several waits on one
    instruction; hoist all but the last into same-engine no-ops inserted
    immediately before it.  The engine executes the no-ops (each blocking
    on one semaphore) then the instruction - identical semantics.

    Matmult/Ldweights get ALL waits hoisted: a wait carried on a PE
    instruction breaks the fill/drain overlap with the previous matmul
    (~210ns per occurrence, once per tile); a NoOp carrying the wait
    dispatches while the previous matmul still streams, so the pipeline
    stays full."""
    import concourse.mybir as mybir

    for fn in nc.m.functions:
        for blk in fn.blocks:
            insts = blk.instructions
            i = 0
            while i < len(insts):
                inst = insts[i]
                si = inst.sync_info
                nhoist = 0
                if si is not None and si.on_wait:
                    if type(inst).__name__ in ("InstMatmult", "InstLdweights"):
                        nhoist = len(si.on_wait)
                    elif len(si.on_wait) > 1:
                        nhoist = len(si.on_wait) - 1
                if nhoist:
                    waits = list(si.on_wait)
                    inst.sync_info = mybir.SyncInfo(
                        on_wait=waits[nhoist:], on_update=list(si.on_update)
                    )
                    for j, w in enumerate(waits[:nhoist]):
                        nop = mybir.InstNoOp(
                            name=nc.get_next_instruction_name(),
                            sync_info=mybir.SyncInfo(on_wait=[w], on_update=[]),
                            bass_nofuse=True,
                            engine=inst.engine,
                        )
                        nc.register_instruction(nop)
                        insts.insert(i + j, nop)
                    i += nhoist
                i += 1


def _build_program(D, H, Bc, with_bias, mm_dtype=None):
    import concourse.bass as bass
    import concourse.mybir as mybir
    import concourse.tile as tile
    from concourse.alu_op_type import AluOpType

    f32 = mybir.dt.float32
    bf16 = mybir.dt.bfloat16
    bf16_mode = (mm_dtype or MM_DTYPE) == "bf16"
    mm_dt = mybir.dt.bfloat16 if bf16_mode else mybir.dt.float32r
    Sig = mybir.ActivationFunctionType.Sigmoid
    Tanh = mybir.ActivationFunctionType.Tanh

    KD = D // P  # K chunks for x-side matmuls
    KH = H // P  # K chunks for h-side matmuls
    TILES = Bc // P
    H2 = H // 2

    nc = bass.Bass()
    # Host-prepacked DRAM layouts: per-partition bytes contiguous, matching
    # the SBUF-resident tiles exactly (one fat descriptor per partition).
    xT_p = nc.declare_dram_parameter("xT", [P, KD * Bc], mm_dt, isOutput=False)
    hT_p = nc.declare_dram_parameter("hT", [P, KH * Bc], mm_dt, isOutput=False)
    hN_p = nc.declare_dram_parameter("hN", [P, TILES * H], bf16, isOutput=False)
    att_p = nc.declare_dram_parameter("att", [P, TILES], f32, isOutput=False)
    wnames = ("wu", "wr", "wh", "uu", "ur", "uh")
    w_p = {n: nc.declare_dram_parameter(n, [P, (KD if n[0] == "w" else KH) * H],
                                        mm_dt, isOutput=False) for n in wnames}
    if with_bias:
        b_p = {n: nc.declare_dram_parameter(n, [P, H], f32, isOutput=False)
               for n in ("bub", "brb", "bhb")}
    out_p = nc.declare_dram_parameter("out", [Bc, H], f32, isOutput=True)

    wview = {n: w_p[n][:].rearrange("ki (ko h) -> ki ko h",
                                    ko=KD if n[0] == "w" else KH) for n in wnames}

    CH_W = _bchunks(Bc)
    HN_W = _hchunks(TILES)
    staged = TILES >= 6 and len(CH_W) >= 2 and CH_W[0] == 2 * P \
        and CH_W[1] == 2 * P

    with tile.TileContext(nc) as tc:
        with (
            tc.tile_pool(name="w", bufs=1) as wpool,
            tc.tile_pool(name="ep", bufs=3) as epool,
            tc.tile_pool(name="ps", bufs=2, space="PSUM") as ppool,
        ):
            w_sb = {n: wpool.tile([P, KD if n[0] == "w" else KH, H], mm_dt,
                                  tag=n, name=f"w_{n}") for n in wnames}
            # One resident tile per batch chunk: DMA src AND dst are then
            # contiguous per partition (2-4KB descriptors; slicing one big
            # [P, KD, Bc] tile gives 512B descriptors and ~1/4 the DMA rate).
            xc_sb = [wpool.tile([P, KD, w], mm_dt, tag=f"xc{c}", name=f"xc{c}")
                     for c, w in enumerate(CH_W)]
            hc_sb = [wpool.tile([P, KH, w], mm_dt, tag=f"hc{c}", name=f"hc{c}")
                     for c, w in enumerate(CH_W)]
            # hN: epilogue h in natural [row, H] layout, resident in 4
            # chunks of 4 tiles (host-packed so partition p holds row
            # t*128+p of each tile -> 4KB contiguous per partition).
            hn_sb = [wpool.tile([P, w, H], bf16, tag=f"hn{c}", name=f"hn{c}")
                     for c, w in enumerate(HN_W)]
            att_sb = wpool.tile([P, TILES], f32, tag="att")

            # tile index -> (chunk, local column offset)
            t2c = {}
            lo = 0
            for c, w in enumerate(CH_W):
                for tt in range(w // P):
                    t2c[(lo + tt * P) // P] = (c, tt * P)
                lo += w

            def operand(t, ki, side):
                c, off = t2c[t]
                sb = xc_sb[c] if side == "x" else hc_sb[c]
                return sb[:, ki, off:off + P]

            def h_nat(t):
                return hn_sb[t // 4][:, t % 4, :]

            # PE warm-up bridge: the HAM clock gate needs ~3.4us of
            # sustained PE activity before it lifts the array clock to
            # 2.4 GHz, and an idle gap resets it. Junk bf16 ldweights +
            # matmuls on memset tiles keep the PE busy from ~7.5us until
            # the first real operands land (~10.5us), so the real matmuls
            # start (nearly) warm. memset on gpsimd so the junk is not
            # gated behind any DMA-issuing engine.
            warm = wpool.tile([P, P], bf16, tag="warm")
            warm2 = wpool.tile([P, H], bf16, tag="warm2")
            nc.gpsimd.memset(warm, 0.0)
            nc.gpsimd.memset(warm2, 0.0)
            for _ in range(N_JUNK_LDW):
                nc.tensor.ldweights(warm)

            psum = {}

            def new_group(t, name):
                psum[name] = ppool.tile([P, H], f32, tag=name,
                                        name=f"p_{name}_{t}")

            # Junk matmuls go to tile 0's zr PSUM bank with start=stop=True;
            # the first real zr matmul re-starts the accumulation group, so
            # the junk results are discarded. Same-engine WAW needs no sync.
            sv = [{} for _ in range(TILES)]
            if staged:
                for t in (0, 1):
                    new_group(t, "zr")
                    sv[t]["zr"] = psum["zr"]
                for _ in range(N_JUNK_MM):
                    nc.tensor.matmul(sv[0]["zr"], warm, warm2,
                                     start=True, stop=True)

            # Direct DMAs on the three DGE queues (sync, scalar, gpsimd),
            # per queue in consumption order with the first-needed piece
            # first. A queue round-robins packets across its outstanding
            # transfers, so co-residency delays completion: the critical
            # path pieces (xc0 / wr q0 / hc0) each lead their own queue.
            def chunk_dma(eng, sbs, view, c):
                lo = sum(CH_W[:c])
                KO = sbs[c].shape[1]
                src = view[:, KO * lo:KO * (lo + CH_W[c])].rearrange(
                    "ki (ko b) -> ki ko b", ko=KO)
                eng.dma_start(sbs[c], src)

            def w_dma(eng, n, piece, npieces):
                KO = w_sb[n].shape[1]
                w = KO // npieces
                sl = slice(piece * w, (piece + 1) * w)
                eng.dma_start(w_sb[n][:, sl], wview[n][:, sl])

            def hn_dma(eng, c):
                lo = sum(HN_W[:c])
                src = hN_p[:, lo * H:(lo + HN_W[c]) * H].rearrange(
                    "p (t h) -> p t h", h=H)
                eng.dma_start(hn_sb[c], src)

            # All weight pieces are ko-HALVES (~256KB, 2KB/partition
            # descriptors): 1KB-descriptor transfers only sustain
            # ~50 GB/s per queue while 2KB run ~120 GB/s. Each queue
            # round-robins packets across its outstanding transfers, so
            # completion order ~= dispatch order only for EQUAL-size
            # pieces in need order — a late-needed piece dispatched early
            # steals bandwidth from every critical piece behind it. Hence
            # strict need-ordering per queue, balanced across queues.
            # sync: xc0, zr h-side, xc1, then uu halves + hN + late x.
            chunk_dma(nc.sync, xc_sb, xT_p[:], 0)
            w_dma(nc.sync, "ur", 0, 2)
            w_dma(nc.sync, "ur", 1, 2)
            if len(CH_W) > 1:
                chunk_dma(nc.sync, xc_sb, xT_p[:], 1)
            w_dma(nc.sync, "uu", 0, 2)
            w_dma(nc.sync, "uu", 1, 2)
            hn_dma(nc.sync, 0)
            for c in range(2, len(CH_W)):
                chunk_dma(nc.sync, xc_sb, xT_p[:], c)
            if len(HN_W) > 1:
                hn_dma(nc.sync, 1)
            # scalar: ONLY the zr x-side halves early — so the queue
            # round-robins over nothing else while they stream. The
            # wh/wu dispatches sit behind an anchor ACT that reads the
            # last column of wr's second half: the engine stalls there
            # until wr is fully landed (~14us), only then enqueues the
            # late pieces (needed ~22-23us).
            w_dma(nc.scalar, "wr", 0, 2)
            w_dma(nc.scalar, "wr", 1, 2)
            anchor = wpool.tile([P, 1], f32, tag="anchor")
            nc.scalar.activation(anchor, w_sb["wr"][:, KD - 1, H - 1:H], Sig)
            w_dma(nc.scalar, "wh", 0, 2)
            w_dma(nc.scalar, "wh", 1, 2)
            w_dma(nc.scalar, "wu", 0, 2)
            w_dma(nc.scalar, "wu", 1, 2)
            # gpsimd: h chunks + hu weight halves + att + late hN chunks.
            chunk_dma(nc.gpsimd, hc_sb, hT_p[:], 0)
            w_dma(nc.gpsimd, "uh", 0, 2)
            w_dma(nc.gpsimd, "uh", 1, 2)
            if len(CH_W) > 1:
                chunk_dma(nc.gpsimd, hc_sb, hT_p[:], 1)
            for c in range(2, len(CH_W)):
                chunk_dma(nc.gpsimd, hc_sb, hT_p[:], c)
            nc.gpsimd.dma_start(att_sb, att_p[:])
            for c in range(2, len(HN_W)):
                hn_dma(nc.gpsimd, c)
            if with_bias:
                b_sb = {}
                for n in ("bub", "brb", "bhb"):
                    t = wpool.tile([P, H], f32, tag=n)
                    nc.scalar.dma_start(t, b_p[n][:])
                    b_sb[n] = t

            GROUP_W = {"zr": ("wr", "ur"), "zu": ("wu", "uu"),
                       "hu": (None, "uh"), "xh": ("wh", None)}

            def mm_piece(t, name, side, kis, csl=None, pt=None):
                """Emit the matmuls of group `name` for tile t restricted
                to `kis` of `side` ('x'/'h') and output columns `csl`.
                start/stop flags derive from the group's first/last matmul."""
                wx, wh_ = GROUP_W[name]
                if pt is None:
                    pt = psum[name]
                first_side = "x" if wx else "h"
                last_side = "h" if wh_ else "x"
                K = KD if side == "x" else KH
                wn = wx if side == "x" else wh_
                for ki in kis:
                    wap = w_sb[wn][:, ki] if csl is None else w_sb[wn][:, ki, csl]
                    nc.tensor.matmul(pt, operand(t, ki, side), wap,
                                     start=side == first_side and ki == 0,
                                     stop=side == last_side and ki == K - 1)

            def mm_groups(t, names):
                for name in names:
                    new_group(t, name)
                    wx, wh_ = GROUP_W[name]
                    if wx:
                        mm_piece(t, name, "x", range(KD))
                    if wh_:
                        mm_piece(t, name, "h", range(KH))

            # Epilogue in three parts so PSUM rings release early. PSUM
            # releasing engines: zr/zu by ACT sigmoid, hu/xh by DVE.
            # Group order zr, hu, xh, zu per tile means the candidate
            # chain (r, r*hu+xh, tanh, -h) completes while the zu matmuls
            # still run; only sig(zu) -> stt -> +h trails the last matmul.
            ep = [{} for _ in range(TILES)]

            def ep_sig_r(t, ps):
                r = epool.tile([P, H], f32, tag="r", name=f"r_{t}")
                if with_bias:
                    zrs = epool.tile([P, H], f32, tag="zrs", name=f"zrs_{t}")
                    nc.vector.tensor_add(zrs, ps["zr"], b_sb["brb"])
                    nc.scalar.activation(r, zrs, Sig)
                else:
                    nc.scalar.activation(r, ps["zr"], Sig)   # releases zr
                ep[t]["r"] = r

            def ep_g1m(t, ps):
                g = epool.tile([P, H], f32, tag="g", name=f"g_{t}")
                nc.vector.tensor_mul(g, ep[t]["r"], ps["hu"])  # releases hu
                ep[t]["g"] = g

            def ep_rest(t, ps):
                bsl = slice(t * P, (t + 1) * P)
                g, h_t = ep[t]["g"], h_nat(t)
                u = epool.tile([P, H], f32, tag="u", name=f"u_{t}")
                d = epool.tile([P, H], f32, tag="d", name=f"d_{t}")
                o = epool.tile([P, H], f32, tag="o", name=f"o_{t}")
                att_c = att_sb[:, t:t + 1]
                nc.vector.tensor_add(g, g, ps["xh"])   # + x @ W_h (rel. xh)
                if with_bias:
                    nc.vector.tensor_add(g, g, b_sb["bhb"])
                nc.scalar.activation(g, g, Tanh)       # hhat
                nc.vector.tensor_sub(d, g, h_t)        # hhat - h
                zsrc = ps["zu"]
                if with_bias:
                    zus = epool.tile([P, H], f32, tag="zus", name=f"zus_{t}")
                    nc.vector.tensor_add(zus, ps["zu"], b_sb["bub"])
                    zsrc = zus
                halves = ps.get("zu2")
                for i, sl in enumerate([slice(0, H2), slice(H2, H)]
                                       if halves else [slice(0, H)]):
                    z = halves[i] if halves else zsrc[:, sl]
                    nc.scalar.activation(u[:, sl], z, Sig)  # releases zu
                    # m = (u * att) * d, fused on DVE
                    nc.vector.scalar_tensor_tensor(d[:, sl], u[:, sl], att_c,
                                                   d[:, sl],
                                                   AluOpType.mult, AluOpType.mult)
                    nc.vector.tensor_add(o[:, sl], d[:, sl], h_t[:, sl])
                    nc.sync.dma_start(out_p[bsl, sl], o[:, sl])

            def last_tile(t, prev):
                """zr/hu/xh as usual; zu as two column-half groups in two
                separate half-bank PSUM tiles (Tile deps are per-tile, so
                separate tiles let half 0's sigmoid overlap half 1's
                matmuls). The g-chain runs before the zu halves; the one
                still-pending ep_rest leads so its DVE/ACT work overlaps
                this tile's matmul stream."""
                ep_rest(prev, sv[prev])
                new_group(t, "zr")
                sv[t]["zr"] = psum["zr"]
                mm_piece(t, "zr", "x", range(KD))
                mm_piece(t, "zr", "h", range(KH))
                ep_sig_r(t, sv[t])
                new_group(t, "hu")
                sv[t]["hu"] = psum["hu"]
                mm_piece(t, "hu", "h", range(KH))
                ep_g1m(t, sv[t])
                new_group(t, "xh")
                sv[t]["xh"] = psum["xh"]
                mm_piece(t, "xh", "x", range(KD))
                ps = sv[t]
                g, h_t = ep[t]["g"], h_nat(t)
                nc.vector.tensor_add(g, g, ps["xh"])
                if with_bias:
                    nc.vector.tensor_add(g, g, b_sb["bhb"])
                nc.scalar.activation(g, g, Tanh)
                d = epool.tile([P, H], f32, tag="d", name=f"d_{t}")
                nc.vector.tensor_sub(d, g, h_t)
                u = epool.tile([P, H], f32, tag="u", name=f"u_{t}")
                o = epool.tile([P, H], f32, tag="o", name=f"o_{t}")
                att_c = att_sb[:, t:t + 1]
                bsl = slice(t * P, (t + 1) * P)
                from concourse.alu_op_type import AluOpType as A
                H4 = H // 4
                for i in range(4):
                    sl = slice(i * H4, (i + 1) * H4)
                    pz = ppool.tile([P, H4], f32, tag="zu", name=f"p_zu_{t}_{i}")
                    mm_piece(t, "zu", "x", range(KD), csl=sl, pt=pz)
                    mm_piece(t, "zu", "h", range(KH), csl=sl, pt=pz)
                    if with_bias:
                        zus = epool.tile([P, H4], f32, tag="zus",
                                         name=f"zus_{t}_{i}")
                        nc.vector.tensor_add(zus, pz, b_sb["bub"][:, sl])
                        nc.scalar.activation(u[:, sl], zus, Sig)
                    else:
                        nc.scalar.activation(u[:, sl], pz, Sig)
                    nc.vector.scalar_tensor_tensor(d[:, sl], u[:, sl], att_c,
                                                   d[:, sl], A.mult, A.mult)
                    nc.vector.tensor_add(o[:, sl], d[:, sl], h_t[:, sl])
                    nc.sync.dma_start(out_p[bsl, sl], o[:, sl])

            if staged:
                # Stages A/B: zr+hu for tiles 0/1 then 2/3, ki-major
                # across each pair in weight-piece arrival order (wr
                # quarters, ur quarters, uh halves). Only 1.5MB of
                # weights is needed for the first 4 tiles' 96 matmuls;
                # wh/wu/uu stream in meanwhile.
                def zr_hu_pair(ta, tb, bridge=0):
                    for t in (ta, tb):
                        if "zr" not in sv[t]:
                            new_group(t, "zr")
                            sv[t]["zr"] = psum["zr"]
                    if bridge:
                        # Pre-allocate ta's hu bank as the junk target for
                        # the wr-half-boundary bridge below.
                        new_group(ta, "hu")
                        sv[ta]["hu"] = psum["hu"]
                    for ki in range(KD):
                        if bridge and ki == KD // 2:
                            # The second wr half usually trails its
                            # consumption point by ~2-3us; junk matmuls
                            # keep the PE array active so the HAM clock
                            # gate doesn't drop back to 1.2 GHz during
                            # the wait (real results unaffected: the
                            # first real hu matmul re-starts the group).
                            for _ in range(bridge):
                                nc.tensor.matmul(sv[ta]["hu"], warm, warm2,
                                                 start=True, stop=True)
                        for t in (ta, tb):
                            mm_piece(t, "zr", "x", (ki,), pt=sv[t]["zr"])
                    for ki in range(KH):
                        for t in (ta, tb):
                            mm_piece(t, "zr", "h", (ki,), pt=sv[t]["zr"])
                    for t in (ta, tb):
                        ep_sig_r(t, sv[t])
                    hk = KH // 2
                    for t in (ta, tb):
                        if not (bridge and t == ta):
                            new_group(t, "hu")
                            sv[t]["hu"] = psum["hu"]
                        mm_piece(t, "hu", "h", range(hk), pt=sv[t]["hu"])
                    for t in (ta, tb):
                        mm_piece(t, "hu", "h", range(hk, KH), pt=sv[t]["hu"])
                    for t in (ta, tb):
                        ep_g1m(t, sv[t])

                def xh_zu_one(t):
                    new_group(t, "xh")
                    sv[t]["xh"] = psum["xh"]
                    hk = KD // 2
                    mm_piece(t, "xh", "x", range(hk), pt=sv[t]["xh"])
                    mm_piece(t, "xh", "x", range(hk, KD), pt=sv[t]["xh"])
                    new_group(t, "zu")
                    sv[t]["zu"] = psum["zu"]
                    mm_piece(t, "zu", "x", range(hk), pt=sv[t]["zu"])
                    mm_piece(t, "zu", "x", range(hk, KD), pt=sv[t]["zu"])
                    kk = KH // 2
                    mm_piece(t, "zu", "h", range(kk), pt=sv[t]["zu"])
                    mm_piece(t, "zu", "h", range(kk, KH), pt=sv[t]["zu"])

                zr_hu_pair(0, 1, bridge=6)
                zr_hu_pair(2, 3)
                xh_zu_one(0)
                ep_rest(0, sv[0])
                xh_zu_one(1)
                ep_rest(1, sv[1])
                xh_zu_one(2)
                ep_rest(2, sv[2])
                xh_zu_one(3)
                start = 4
            else:
                start = 0
            # Main loop: tile-major, tile t's iteration LEADS with the
            # deferred ep_rest of tile t-1: its ACT ops (tanh, sig u) have
            # ready inputs and run before sig_r(t) (whose input only lands
            # 1.7us into the iteration), so the ACT queue never
            # head-of-line blocks a PSUM-slot release, and DVE does the
            # rest work during this tile's matmul stream.
            for t in range(start, TILES - 1 if staged else TILES):
                if staged:
                    ep_rest(t - 1, sv[t - 1])
                new_group(t, "zr")
                sv[t] = {"zr": psum["zr"]}
                mm_piece(t, "zr", "x", range(KD))
                mm_piece(t, "zr", "h", range(KH))
                ep_sig_r(t, sv[t])
                new_group(t, "hu")
                sv[t]["hu"] = psum["hu"]
                mm_piece(t, "hu", "h", range(KH))
                ep_g1m(t, sv[t])
                new_group(t, "xh")
                sv[t]["xh"] = psum["xh"]
                mm_piece(t, "xh", "x", range(KD))
                new_group(t, "zu")
                sv[t]["zu"] = psum["zu"]
                mm_piece(t, "zu", "x", range(KD))
                mm_piece(t, "zu", "h", range(KH))
                if not staged:
                    ep_rest(t, sv[t])
            if staged:
                last_tile(TILES - 1, TILES - 2)

    _split_multi_waits(nc)
    return nc


def check_waits(nc):
    """Matmults and Drains may carry at most 1 sync wait on walrus; other
    instruction classes tolerate more (walrus splits them itself)."""
    bad = []
    for fn in nc.m.functions:
        for blk in fn.blocks:
            for inst in blk.instructions:
                si = inst.sync_info
                nw = len(si.on_wait) if si else 0
                kind = type(inst).__name__
                if nw > 1:
                    bad.append((inst.name, kind, nw))
    return bad


def _get_program(D, H, Bc, with_bias):
    key = (D, H, Bc, with_bias, MM_DTYPE)
    if key not in _PROGRAM_CACHE:
        nc = _build_program(D, H, Bc, with_bias)
        bad = check_waits(nc)
        if bad:
            raise RuntimeError(f"instructions over the sync-wait limit: {bad}")
        _PROGRAM_CACHE[key] = nc
    return _PROGRAM_CACHE[key]


def _np32(a):
    return np.ascontiguousarray(np.asarray(a, dtype=np.float32))


def _bf16():
    import ml_dtypes

    return ml_dtypes.bfloat16


def _mm_np_dtype():
    return _bf16() if MM_DTYPE == "bf16" else np.float32


def _pack_bT(a, Bc, K, mmdt):
    """[Bc, K] activations -> [128, KO*Bc] with per-partition layout
    [chunk][ko][b_local] (chunk-major, matching the per-chunk DMAs)."""
    KO = K // P
    parts, lo = [], 0
    for w in _bchunks(Bc):
        blk = a[lo:lo + w].reshape(w, KO, P).transpose(2, 1, 0)  # [ki, ko, b]
        parts.append(blk.reshape(P, KO * w))
        lo += w
    return np.ascontiguousarray(np.concatenate(parts, axis=1).astype(mmdt))


def _pack_hN(a, Bc, H):
    """[Bc, H] -> [128, T*H] bf16: partition p holds row t*128+p of each
    tile t, so a chunk of tiles is one fat contiguous descriptor."""
    T = Bc // P
    out = a.reshape(T, P, H).transpose(1, 0, 2).reshape(P, T * H)
    return np.ascontiguousarray(out.astype(_bf16()))


def _pack_w(w, mmdt):
    """[K, H] weight -> [128, KO*H] with per-partition layout [ko, h]."""
    K, H = w.shape
    out = w.reshape(K // P, P, H).transpose(1, 0, 2)
    return np.ascontiguousarray(out.reshape(P, -1).astype(mmdt))


def _prepare(x, att_score, hidden, W_u, U_u, b_u, W_r, U_r, b_r, W_h, U_h, b_h):
    x = _np32(x)
    att_score = _np32(att_score)
    hidden = _np32(hidden)
    B, D = x.shape
    H = hidden.shape[1]
    assert B % (NCORES * P) == 0 and D % P == 0 and H % P == 0
    Bc = B // NCORES
    mmdt = _mm_np_dtype()

    weights = {
        "wu": _np32(W_u), "wr": _np32(W_r), "wh": _np32(W_h),
        "uu": _np32(U_u), "ur": _np32(U_r), "uh": _np32(U_h),
    }
    biases = [_np32(b_u), _np32(b_r), _np32(b_h)]
    with_bias = any(np.any(b) for b in biases)
    packed_w = {k: _pack_w(v, mmdt) for k, v in weights.items()}

    in_maps = []
    for c in range(NCORES):
        sl = slice(c * Bc, (c + 1) * Bc)
        xs, hs, at = x[sl], hidden[sl], att_score[sl]
        m = {
            "xT": _pack_bT(xs, Bc, D, mmdt),
            "hT": _pack_bT(hs, Bc, H, mmdt),
            "hN": _pack_hN(hs, Bc, H),
            "att": np.ascontiguousarray(at.reshape(Bc // P, P).T),
        }
        m.update(packed_w)
        if with_bias:
            m["bub"] = np.ascontiguousarray(np.broadcast_to(biases[0], (P, H)))
            m["brb"] = np.ascontiguousarray(np.broadcast_to(biases[1], (P, H)))
            m["bhb"] = np.ascontiguousarray(np.broadcast_to(biases[2], (P, H)))
        in_maps.append(m)

    nc = _get_program(D, H, Bc, with_bias)
    return nc, in_maps


def _run(inputs, trace=False, **trace_kwargs):
    from concourse.bass_utils import run_bass_kernel_spmd

    nc, in_maps = _prepare(**inputs)
    res = run_bass_kernel_spmd(nc, in_maps, list(range(NCORES)), trace=trace,
                               **trace_kwargs)
    out = np.concatenate([res.results[i]["out"] for i in range(NCORES)], axis=0)
    return out, res


def kernel(**inputs):
    out, _ = _run(inputs, trace=False)
    return out
